# revision 1
# baseline (speedup 1.0000x reference)
"""DTML model kernel for 8 Trainium2 NeuronCores.

Two SPMD launches:
  A: one attention-LSTM per core. Cores 0-3 run the "stock" LSTM on batch
     quarters (B=128 each), cores 4-7 the "macro" LSTM on the same batch
     quarters. Identical program; per-core inputs pick weights/batch slice.
     Per core: input transform (tanh(x@W.T+b), transposed layout), 256-step
     LSTM recurrence, dot-attention over time. Output ctx [128, 256].
  B: post-phase (normalize + 32-token MHA + MLP + head), data-parallel over
     batch (64 per core), feature-on-partitions layout.

Launch A layouts (per core, B=128, T=256, D=128, H=256):
  gates [B part, 4H free], gate column order [f|i|g|o] (weights permuted on
  host). Recurrent matmul out = lhsT.T @ rhs: lhsT = data (stockinT_t, hT
  tiles, ones-row for the bias), rhs = weight rows streaming. h transposed
  each step on PE for the next step's lhsT.
"""

import sys

sys.path.insert(0, "/opt/trn_rl_repo")

from contextlib import ExitStack

import numpy as np

import concourse.bacc as bacc
import concourse.mybir as mybir
from concourse.masks import make_identity
from concourse.tile import TileContext
from concourse.bass_utils import run_bass_kernel_spmd

FP = mybir.dt.float32
AX = mybir.AxisListType
OP = mybir.AluOpType
AF = mybir.ActivationFunctionType

B, S, D = 512, 256, 128
H = 256
NSTOCK = 32
NHEADS = 8
DH = H // NHEADS
BC = B // 8      # 64  batch per core in launch B
BL = B // 4      # 128 batch per core in launch A
NTOK = BC * NSTOCK  # 2048


# ----------------------------------------------------------------------------
# Launch A
# ----------------------------------------------------------------------------

def build_lstm_program(T=S):
    nc = bacc.Bacc("TRN2", target_bir_lowering=False, debug=False)

    x_d = nc.dram_tensor("x", [BL, T * D], FP, kind="ExternalInput")
    wtr_d = nc.dram_tensor("wtr", [D, D], FP, kind="ExternalInput")      # tr_w.T
    btr_d = nc.dram_tensor("btr", [D, 1], FP, kind="ExternalInput")
    wih_d = nc.dram_tensor("wih", [D, 4 * H], FP, kind="ExternalInput")  # wih[perm].T
    whh_d = nc.dram_tensor("whh", [H, 4 * H], FP, kind="ExternalInput")  # whh[perm].T
    bias_d = nc.dram_tensor("bias", [1, 4 * H], FP, kind="ExternalInput")
    ctx_d = nc.dram_tensor("ctx", [BL, H], FP, kind="ExternalOutput")
    den_d = nc.dram_tensor("den", [BL, 1], FP, kind="ExternalOutput")

    sinT_d = nc.dram_tensor("sinT_d", [D, T * BL], FP)
    hs_d = nc.dram_tensor("hs_d", [BL, T * H], FP)

    with TileContext(nc) as tc, ExitStack() as big:
        consts = big.enter_context(tc.tile_pool(name="consts", bufs=1))
        ident = consts.tile([128, 128], FP, name="ident")
        make_identity(nc, ident[:])
        wtr = consts.tile([D, D], FP, name="wtr")
        nc.sync.dma_start(out=wtr[:], in_=wtr_d.ap())
        btr = consts.tile([D, 1], FP, name="btr")
        nc.sync.dma_start(out=btr[:], in_=btr_d.ap())
        wih = consts.tile([D, 4 * H], FP, name="wih")
        nc.sync.dma_start(out=wih[:], in_=wih_d.ap())
        whh0 = consts.tile([128, 4 * H], FP, name="whh0")
        nc.sync.dma_start(out=whh0[:], in_=whh_d.ap()[0:128, :])
        whh1 = consts.tile([128, 4 * H], FP, name="whh1")
        nc.sync.dma_start(out=whh1[:], in_=whh_d.ap()[128:256, :])
        biasr = consts.tile([1, 4 * H], FP, name="biasr")
        nc.sync.dma_start(out=biasr[:], in_=bias_d.ap())
        ones1 = consts.tile([1, 128], FP, name="ones1")
        nc.gpsimd.memset(ones1[:], 1.0)

        # ---- pre-phase: sinT_t = tanh(wtr.T @ x_tT + btr), stored to DRAM
        PRE = 4
        with tc.tile_pool(name="pre_sb", bufs=3) as pre_sb, \
             tc.tile_pool(name="pre_ps", bufs=2, space="PSUM") as pre_ps, \
             tc.tile_pool(name="pre_o", bufs=3) as pre_o:
            for t0 in range(0, T, PRE):
                xc = pre_sb.tile([BL, PRE * D], FP, name="xc")
                nc.sync.dma_start(
                    out=xc[:], in_=x_d.ap()[:, t0 * D:(t0 + PRE) * D])
                xT_ps = pre_ps.tile([128, PRE * 128], FP, name="xT_ps")
                xT_sb = pre_sb.tile([128, PRE * 128], FP, name="xT_sb")
                tf_ps = pre_ps.tile([128, PRE * 128], FP, name="tf_ps")
                sc = pre_o.tile([128, PRE * 128], FP, name="sc")
                for j in range(PRE):
                    sl = slice(j * 128, (j + 1) * 128)
                    nc.tensor.transpose(xT_ps[:, sl], xc[:, sl], ident[:])
                    nc.vector.tensor_copy(xT_sb[:, sl], xT_ps[:, sl])
                    nc.tensor.matmul(tf_ps[:, sl], wtr[:], xT_sb[:, sl],
                                     start=True, stop=True)
                nc.scalar.activation(sc[:], tf_ps[:], AF.Tanh, bias=btr[:])
                nc.sync.dma_start(
                    out=sinT_d.ap()[:, t0 * BL:(t0 + PRE) * BL], in_=sc[:])

        # ---- recurrence
        state = big.enter_context(tc.tile_pool(name="state", bufs=1))
        hT = state.tile([128, 2 * BL], FP, name="hT")
        c = state.tile([BL, H], FP, name="c")

        rec_sin = big.enter_context(tc.tile_pool(name="rec_sin", bufs=4))
        rec_ps = big.enter_context(tc.tile_pool(name="rec_ps", bufs=2, space="PSUM"))
        rec_tr = big.enter_context(tc.tile_pool(name="rec_tr", bufs=2, space="PSUM"))
        rec_sb = big.enter_context(tc.tile_pool(name="rec_sb", bufs=3))
        rec_h = big.enter_context(tc.tile_pool(name="rec_h", bufs=3))

        h_prev = None
        for t in range(T):
            sin = rec_sin.tile([128, BL], FP, name="sin")
            nc.sync.dma_start(
                out=sin[:], in_=sinT_d.ap()[:, t * BL:(t + 1) * BL])
            gates = rec_ps.tile([BL, 4 * H], FP, name="gates")
            for bk in range(2):
                ns = slice(bk * 512, (bk + 1) * 512)
                nc.tensor.matmul(gates[:, ns], sin[:], wih[:, ns],
                                 start=True, stop=False)
                nc.tensor.matmul(gates[:, ns], ones1[:], biasr[:, ns],
                                 start=False, stop=(t == 0))
            if t > 0:
                trp = rec_tr.tile([128, 2 * BL], FP, name="trp")
                for k in range(2):
                    sl = slice(k * 128, (k + 1) * 128)
                    nc.tensor.transpose(trp[:, sl], h_prev[:, sl], ident[:])
                    nc.vector.tensor_copy(hT[:, sl], trp[:, sl])
                for bk in range(2):
                    ns = slice(bk * 512, (bk + 1) * 512)
                    nc.tensor.matmul(gates[:, ns], hT[:, 0:BL], whh0[:, ns],
                                     start=False, stop=False)
                    nc.tensor.matmul(gates[:, ns], hT[:, BL:2 * BL],
                                     whh1[:, ns], start=False, stop=True)
            # gate cols [f|i|g|o]
            sfi = rec_sb.tile([BL, 512], FP, name="sfi")
            nc.scalar.activation(sfi[:], gates[:, 0:512], AF.Sigmoid)
            tg = rec_sb.tile([BL, H], FP, name="tg")
            nc.scalar.activation(tg[:], gates[:, 512:768], AF.Tanh)
            so = rec_sb.tile([BL, H], FP, name="so")
            nc.scalar.activation(so[:], gates[:, 768:1024], AF.Sigmoid)
            if t == 0:
                nc.vector.tensor_tensor(c[:], sfi[:, 256:512], tg[:], OP.mult)
            else:
                m2 = rec_sb.tile([BL, H], FP, name="m2")
                nc.vector.tensor_tensor(m2[:], sfi[:, 0:256], c[:], OP.mult)
                m1 = rec_sb.tile([BL, H], FP, name="m1")
                nc.vector.tensor_tensor(m1[:], sfi[:, 256:512], tg[:], OP.mult)
                nc.vector.tensor_tensor(c[:], m1[:], m2[:], OP.add)
            tct = rec_sb.tile([BL, H], FP, name="tct")
            nc.scalar.activation(tct[:], c[:], AF.Tanh)
            h = rec_h.tile([BL, H], FP, name="h")
            nc.vector.tensor_tensor(h[:], so[:], tct[:], OP.mult)
            nc.sync.dma_start(out=hs_d.ap()[:, t * H:(t + 1) * H], in_=h[:])
            h_prev = h

        # ---- attention over time
        att = big.enter_context(tc.tile_pool(name="att", bufs=1))
        hlast = att.tile([BL, H], FP, name="hlast")
        nc.vector.tensor_copy(hlast[:], h_prev[:])
        score = att.tile([BL, T], FP, name="score")
        ACH = min(8, T)
        with tc.tile_pool(name="att_in", bufs=3) as att_in, \
             tc.tile_pool(name="att_junk", bufs=4) as att_junk:
            for t0 in range(0, T, ACH):
                hc = att_in.tile([BL, ACH * H], FP, name="hc")
                nc.sync.dma_start(
                    out=hc[:], in_=hs_d.ap()[:, t0 * H:(t0 + ACH) * H])
                for j in range(ACH):
                    jk = att_junk.tile([BL, H], FP, name="jk")
                    nc.vector.scalar_tensor_tensor(
                        jk[:], hc[:, j * H:(j + 1) * H], 0.0, hlast[:],
                        OP.bypass, OP.mult,
                        accum_out=score[:, t0 + j:t0 + j + 1])
            denom = att.tile([BL, 1], FP, name="denom")
            nc.vector.tensor_reduce(denom[:], score[:], AX.X, OP.add)
            nc.sync.dma_start(out=den_d.ap(), in_=denom[:])
            ctxA = att.tile([BL, H], FP, name="ctxA")
            nc.vector.memset(ctxA[:], 0.0)
            ctxB = att.tile([BL, H], FP, name="ctxB")
            nc.gpsimd.memset(ctxB[:], 0.0)
            for t0 in range(0, T, ACH):
                hc = att_in.tile([BL, ACH * H], FP, name="hc2")
                nc.sync.dma_start(
                    out=hc[:], in_=hs_d.ap()[:, t0 * H:(t0 + ACH) * H])
                for j in range(ACH):
                    t = t0 + j
                    acc = ctxA if j % 2 == 0 else ctxB
                    nc.vector.scalar_tensor_tensor(
                        acc[:], hc[:, j * H:(j + 1) * H],
                        score[:, t:t + 1], acc[:], OP.mult, OP.add)
            ctx = att.tile([BL, H], FP, name="ctx")
            nc.vector.tensor_tensor(ctx[:], ctxA[:], ctxB[:], OP.add)
            nc.sync.dma_start(out=ctx_d.ap(), in_=ctx[:])

    nc.finalize()
    return nc


_PERM = None


def _gate_perm():
    global _PERM
    if _PERM is None:
        i0 = np.arange(H)
        _PERM = np.concatenate([H + i0, i0, 2 * H + i0, 3 * H + i0])  # f,i,g,o
    return _PERM


def _lstm_core_inputs(x_np, tr_w, tr_b, wih, whh, bih, bhh, T):
    perm = _gate_perm()
    f32 = lambda a: np.ascontiguousarray(np.asarray(a, np.float32))
    return {
        "x": f32(x_np.reshape(BL, T * D)),
        "wtr": f32(np.asarray(tr_w).T),
        "btr": f32(np.asarray(tr_b).reshape(D, 1)),
        "wih": f32(np.asarray(wih)[perm].T),
        "whh": f32(np.asarray(whh)[perm].T),
        "bias": f32((np.asarray(bih) + np.asarray(bhh))[perm].reshape(1, 4 * H)),
    }


# ----------------------------------------------------------------------------
# Launch B
# ----------------------------------------------------------------------------

def build_post_program():
    nc = bacc.Bacc("TRN2", target_bir_lowering=False, debug=False)

    uT_d = nc.dram_tensor("uT", [H, BC], FP, kind="ExternalInput")   # ((cm-mean)/std).T
    vT_d = nc.dram_tensor("vT", [H, BC], FP, kind="ExternalInput")   # (mw*mc).T
    nwT_d = nc.dram_tensor("nwT", [H, NSTOCK], FP, kind="ExternalInput")
    nbT_d = nc.dram_tensor("nbT", [H, NSTOCK], FP, kind="ExternalInput")
    ipwT_d = nc.dram_tensor("ipwT", [H, 3 * H], FP, kind="ExternalInput")  # q-cols pre-scaled
    ipb_d = nc.dram_tensor("ipb", [32, 16], FP, kind="ExternalInput")      # [32d, q8|k8]
    opwT_d = nc.dram_tensor("opwT", [H, H], FP, kind="ExternalInput")
    opb_d = nc.dram_tensor("opb", [128, 2], FP, kind="ExternalInput")      # bo' per-ptile
    w1T_d = nc.dram_tensor("w1T", [H, 4 * H], FP, kind="ExternalInput")
    b1_d = nc.dram_tensor("b1", [128, 8], FP, kind="ExternalInput")
    w2T_d = nc.dram_tensor("w2T", [4 * H, H], FP, kind="ExternalInput")
    b2_d = nc.dram_tensor("b2", [128, 2], FP, kind="ExternalInput")
    fw_d = nc.dram_tensor("fw", [H, 1], FP, kind="ExternalInput")          # final_w/NSTOCK
    out_d = nc.dram_tensor("out", [BC, 1], FP, kind="ExternalOutput")

    NCH = 512

    with TileContext(nc) as tc, ExitStack() as big:
        P = big.enter_context(tc.tile_pool(name="main", bufs=1))
        ps = big.enter_context(tc.tile_pool(name="ps", bufs=2, space="PSUM"))

        def load(name, dram, shape):
            tl = P.tile(shape, FP, name=name)
            nc.sync.dma_start(out=tl[:], in_=dram.ap())
            return tl

        ipb = load("ipb", ipb_d, [32, 16])
        opb = load("opb", opb_d, [128, 2])
        b1 = load("b1", b1_d, [128, 8])
        b2 = load("b2", b2_d, [128, 2])
        fw = P.tile([128, 2], FP, name="fw")
        nc.sync.dma_start(
            out=fw[:].rearrange("p (k o) -> p k o", k=2),
            in_=fw_d.ap().rearrange("(k p) o -> p k o", p=128))
        # weights, stored as [128, ktiles*cols] blocks
        ipwT = P.tile([128, 2 * 768], FP, name="ipwT")
        nc.sync.dma_start(
            out=ipwT[:].rearrange("p (k n) -> p k n", k=2),
            in_=ipwT_d.ap().rearrange("(k p) n -> p k n", p=128))
        opwT = P.tile([128, 2 * 256], FP, name="opwT")
        nc.sync.dma_start(
            out=opwT[:].rearrange("p (k n) -> p k n", k=2),
            in_=opwT_d.ap().rearrange("(k p) n -> p k n", p=128))
        w1T = P.tile([128, 2 * 1024], FP, name="w1T")
        nc.sync.dma_start(
            out=w1T[:].rearrange("p (k n) -> p k n", k=2),
            in_=w1T_d.ap().rearrange("(k p) n -> p k n", p=128))
        w2T = P.tile([128, 8 * 256], FP, name="w2T")
        nc.sync.dma_start(
            out=w2T[:].rearrange("p (k n) -> p k n", k=8),
            in_=w2T_d.ap().rearrange("(k p) n -> p k n", p=128))

        uT2 = P.tile([128, 2 * BC], FP, name="uT2")
        nc.sync.dma_start(
            out=uT2[:].rearrange("p (k n) -> p k n", k=2),
            in_=uT_d.ap().rearrange("(k p) n -> p k n", p=128))
        vT2 = P.tile([128, 2 * BC], FP, name="vT2")
        nc.sync.dma_start(
            out=vT2[:].rearrange("p (k n) -> p k n", k=2),
            in_=vT_d.ap().rearrange("(k p) n -> p k n", p=128))
        nwT2 = P.tile([128, 2 * NSTOCK], FP, name="nwT2")
        nc.sync.dma_start(
            out=nwT2[:].rearrange("p (k n) -> p k n", k=2),
            in_=nwT_d.ap().rearrange("(k p) n -> p k n", p=128))
        nbT2 = P.tile([128, 2 * NSTOCK], FP, name="nbT2")
        nc.sync.dma_start(
            out=nbT2[:].rearrange("p (k n) -> p k n", k=2),
            in_=nbT_d.ap().rearrange("(k p) n -> p k n", p=128))

        # ---- mlT [128, 2*NTOK], col = b*32+s
        mlT = P.tile([128, 2 * NTOK], FP, name="mlT")
        tmp = P.tile([128, NTOK], FP, name="tmp")
        for k in range(2):
            msl = slice(k * NTOK, (k + 1) * NTOK)
            nc.vector.tensor_tensor(
                tmp[:].rearrange("p (b s) -> p b s", s=NSTOCK),
                nwT2[:, k * NSTOCK:(k + 1) * NSTOCK].unsqueeze(1)
                    .broadcast_to([128, BC, NSTOCK]),
                uT2[:, k * BC:(k + 1) * BC].unsqueeze(2)
                    .broadcast_to([128, BC, NSTOCK]),
                OP.mult)
            nc.vector.tensor_tensor(
                mlT[:, msl].rearrange("p (b s) -> p b s", s=NSTOCK),
                nbT2[:, k * NSTOCK:(k + 1) * NSTOCK].unsqueeze(1)
                    .broadcast_to([128, BC, NSTOCK]),
                vT2[:, k * BC:(k + 1) * BC].unsqueeze(2)
                    .broadcast_to([128, BC, NSTOCK]),
                OP.add)
            nc.vector.tensor_tensor(mlT[:, msl], mlT[:, msl], tmp[:], OP.add)

        # ---- MHA in b-chunks
        attnT = P.tile([128, 2 * NTOK], FP, name="attnT")
        BCH = 8
        with tc.tile_pool(name="mha_sb", bufs=1) as mha_sb, \
             tc.tile_pool(name="mha_ps", bufs=2, space="PSUM") as mha_ps:
            for b0 in range(0, BC, BCH):
                # q32/k32 chunks [32 d-part, (hd, bw, s)]
                q32 = mha_sb.tile([32, NHEADS * BCH * 32], FP, name="q32")
                k32 = mha_sb.tile([32, NHEADS * BCH * 32], FP, name="k32")
                for hd in range(NHEADS):
                    for qk in range(2):
                        qp = mha_ps.tile([32, BCH * 32], FP, name="qp", tag="mps")
                        for k in range(2):
                            nc.tensor.matmul(
                                qp[:],
                                ipwT[:, k * 768 + qk * 256 + hd * 32:
                                     k * 768 + qk * 256 + (hd + 1) * 32],
                                mlT[:, k * NTOK + b0 * 32:
                                    k * NTOK + (b0 + BCH) * 32],
                                start=(k == 0), stop=(k == 1))
                        dst = q32 if qk == 0 else k32
                        nc.vector.tensor_scalar(
                            dst[:, hd * BCH * 32:(hd + 1) * BCH * 32], qp[:],
                            ipb[0:32, qk * 8 + hd:qk * 8 + hd + 1], None,
                            OP.add)
                # v_tok chunk [32, BCH*256], col = (b-b0)*256 + hd*32 + d
                v_tok = mha_sb.tile([32, BCH * 256], FP, name="v_tok")
                for bi in range(0, BCH, 2):
                    vp = mha_ps.tile([32, 512], FP, name="vp", tag="mps")
                    for bj in range(2):
                        b = b0 + bi + bj
                        for k in range(2):
                            nc.tensor.matmul(
                                vp[:, bj * 256:(bj + 1) * 256],
                                mlT[:, k * NTOK + b * 32:k * NTOK + (b + 1) * 32],
                                ipwT[:, k * 768 + 512:k * 768 + 768],
                                start=(k == 0), stop=(k == 1))
                    nc.vector.tensor_copy(
                        v_tok[:, bi * 256:(bi + 2) * 256], vp[:])
                # scoresT -> exp, esc chunk [32, BCH*256], col=(b-b0)*256+hd*32+s
                esc = mha_sb.tile([32, BCH * 256], FP, name="esc")
                for bi in range(0, BCH, 2):
                    sp = mha_ps.tile([32, 512], FP, name="sp", tag="mps")
                    for bj in range(2):
                        bw = bi + bj
                        for hd in range(NHEADS):
                            co = hd * BCH * 32 + bw * 32
                            nc.tensor.matmul(
                                sp[0:32, bj * 256 + hd * 32:bj * 256 + (hd + 1) * 32],
                                k32[0:32, co:co + 32],
                                q32[0:32, co:co + 32],
                                start=True, stop=True)
                    nc.scalar.activation(
                        esc[:, bi * 256:(bi + 2) * 256], sp[:], AF.Exp)
                # ssum over k (partitions) -> [1, BCH*256] -> normalize esc
                ssum = mha_sb.tile([1, BCH * 256], FP, name="ssum")
                nc.gpsimd.tensor_reduce(ssum[:], esc[:], AX.C, OP.add)
                s128 = mha_sb.tile([128, BCH * 2], FP, name="s128")
                nc.sync.dma_start(
                    out=s128[:],
                    in_=ssum[0:1, :].rearrange("o (p f) -> o p f", p=128))
                l128 = mha_sb.tile([128, BCH * 2], FP, name="l128")
                nc.scalar.activation(l128[:], s128[:], AF.Ln)
                r128 = mha_sb.tile([128, BCH * 2], FP, name="r128")
                nc.scalar.activation(r128[:], l128[:], AF.Exp, scale=-1.0)
                rflat = mha_sb.tile([1, BCH * 256], FP, name="rflat")
                nc.sync.dma_start(
                    out=rflat[0:1, :].rearrange("o (p f) -> o p f", p=128),
                    in_=r128[:])
                rrep = mha_sb.tile([32, BCH * 256], FP, name="rrep")
                nc.gpsimd.partition_broadcast(rrep[:], rflat[0:1, :])
                nc.vector.tensor_tensor(esc[:], esc[:], rrep[:], OP.mult)
                # AV: attnT chunk, 2 psum tiles per 4 b's
                for bi in range(0, BCH, 4):
                    for hf in range(2):
                        ap_ps = mha_ps.tile([128, 128], FP, name="ap_ps")
                        for bj in range(4):
                            b = b0 + bi + bj
                            for hq in range(4):
                                hd = hf * 4 + hq
                                col = (bi + bj) * 256 + hd * 32
                                nc.tensor.matmul(
                                    ap_ps[hq * 32:(hq + 1) * 32,
                                          bj * 32:(bj + 1) * 32],
                                    v_tok[0:32, col:col + 32],
                                    esc[0:32, col:col + 32],
                                    start=True, stop=True,
                                    tile_position=(0, hq * 32))
                        nc.vector.tensor_copy(
                            attnT[:, hf * NTOK + (b0 + bi) * 32:
                                  hf * NTOK + (b0 + bi + 4) * 32], ap_ps[:])

        # ---- att_outT + residual -> h1T
        h1T = P.tile([128, 2 * NTOK], FP, name="h1T")
        for m in range(2):
            for n0 in range(0, NTOK, NCH):
                op_ps = ps.tile([128, NCH], FP, name="op_ps", tag="mmps")
                for k in range(2):
                    nc.tensor.matmul(
                        op_ps[:], opwT[:, k * 256 + m * 128:k * 256 + (m + 1) * 128],
                        attnT[:, k * NTOK + n0:k * NTOK + n0 + NCH],
                        start=(k == 0), stop=(k == 1))
                nc.vector.scalar_tensor_tensor(
                    h1T[:, m * NTOK + n0:m * NTOK + n0 + NCH], op_ps[:],
                    opb[:, m:m + 1], mlT[:, m * NTOK + n0:m * NTOK + n0 + NCH],
                    OP.add, OP.add)

        # ---- MLP fused over n-chunks; outT = tanh(h1T + mlp)
        outT = P.tile([128, 2 * NTOK], FP, name="outT")
        with tc.tile_pool(name="mid_sb", bufs=2) as mid_sb:
            for n0 in range(0, NTOK, NCH):
                mid = mid_sb.tile([128, 8 * NCH], FP, name="mid")
                for m in range(8):
                    mp = ps.tile([128, NCH], FP, name="mp", tag="mmps")
                    for k in range(2):
                        nc.tensor.matmul(
                            mp[:], w1T[:, k * 1024 + m * 128:k * 1024 + (m + 1) * 128],
                            h1T[:, k * NTOK + n0:k * NTOK + n0 + NCH],
                            start=(k == 0), stop=(k == 1))
                    nc.vector.tensor_scalar(
                        mid[:, m * NCH:(m + 1) * NCH], mp[:],
                        b1[:, m:m + 1], 0.0, OP.add, op1=OP.max)
                for m in range(2):
                    op2 = ps.tile([128, NCH], FP, name="op2", tag="mmps")
                    for k in range(8):
                        nc.tensor.matmul(
                            op2[:], w2T[:, k * 256 + m * 128:k * 256 + (m + 1) * 128],
                            mid[:, k * NCH:(k + 1) * NCH],
                            start=(k == 0), stop=(k == 7))
                    pre = mid_sb.tile([128, NCH], FP, name="pre")
                    nc.vector.scalar_tensor_tensor(
                        pre[:], op2[:], b2[:, m:m + 1],
                        h1T[:, m * NTOK + n0:m * NTOK + n0 + NCH],
                        OP.add, OP.add)
                    nc.scalar.activation(
                        outT[:, m * NTOK + n0:m * NTOK + n0 + NCH], pre[:],
                        AF.Tanh)

        # ---- pool over s, final head
        pooledT = P.tile([128, 2 * BC], FP, name="pooledT")
        for k in range(2):
            nc.vector.tensor_reduce(
                pooledT[:, k * BC:(k + 1) * BC],
                outT[:, k * NTOK:(k + 1) * NTOK].rearrange(
                    "p (b s) -> p b s", s=NSTOCK),
                AX.X, OP.add)
        fin_ps = ps.tile([BC, 1], FP, name="fin_ps", tag="mmps")
        for k in range(2):
            nc.tensor.matmul(fin_ps[:], pooledT[:, k * BC:(k + 1) * BC],
                             fw[:, k:k + 1],
                             start=(k == 0), stop=(k == 1))
        fin = P.tile([BC, 1], FP, name="fin")
        nc.vector.tensor_copy(fin[:], fin_ps[:])
        nc.sync.dma_start(out=out_d.ap(), in_=fin[:])

    nc.finalize()
    return nc


def _post_core_inputs(cm_b, mc_b, inputs):
    f32 = lambda a: np.ascontiguousarray(np.asarray(a, np.float32))
    mw = float(np.asarray(inputs["macro_weight"]).reshape(-1)[0])
    mean = cm_b.mean(1, keepdims=True)
    std = cm_b.std(1, keepdims=True, ddof=1) + 1e-8
    uT = ((cm_b - mean) / std).T
    vT = (mc_b * mw).T
    ipw = np.asarray(inputs["in_proj_w"], np.float32)
    ipb = np.asarray(inputs["in_proj_b"], np.float32)
    opw = np.asarray(inputs["out_proj_w"], np.float32)
    opb = np.asarray(inputs["out_proj_b"], np.float32)
    qsc = 1.0 / np.sqrt(DH)
    ipwT = ipw.T.copy()
    ipwT[:, 0:H] *= qsc
    ipb_eff = ipb.copy()
    ipb_eff[0:H] *= qsc
    opb_eff = opb + ipb[2 * H:] @ opw.T
    return {
        "uT": f32(uT),
        "vT": f32(vT),
        "nwT": f32(np.asarray(inputs["norm_weight"]).T),
        "nbT": f32(np.asarray(inputs["norm_bias"]).T),
        "ipwT": f32(ipwT),
        "ipb": f32(ipb_eff[0:2 * H].reshape(2, 8, 32).transpose(2, 0, 1).reshape(32, 16)),
        "opwT": f32(opw.T),
        "opb": f32(opb_eff.reshape(2, 128).T),
        "w1T": f32(np.asarray(inputs["mlp_w1"]).T),
        "b1": f32(np.asarray(inputs["mlp_b1"]).reshape(8, 128).T),
        "w2T": f32(np.asarray(inputs["mlp_w2"]).T),
        "b2": f32(np.asarray(inputs["mlp_b2"]).reshape(2, 128).T),
        "fw": f32((np.asarray(inputs["final_w"]).reshape(H) / NSTOCK).reshape(H, 1)),
    }


# ----------------------------------------------------------------------------
# host orchestration
# ----------------------------------------------------------------------------

_progs = {}


def run_lstm_launch(inputs, T=S, trace=False):
    if ("lstm", T) not in _progs:
        _progs[("lstm", T)] = build_lstm_program(T)
    nc_a = _progs[("lstm", T)]
    x = np.asarray(inputs["x"], np.float32)
    in_maps = []
    for core in range(8):
        q = core % 4
        xb = x[q * BL:(q + 1) * BL]
        if core < 4:
            m = _lstm_core_inputs(xb, inputs["stock_tr_w"], inputs["stock_tr_b"],
                                  inputs["s_wih"], inputs["s_whh"],
                                  inputs["s_bih"], inputs["s_bhh"], T)
        else:
            m = _lstm_core_inputs(xb, inputs["macro_tr_w"], inputs["macro_tr_b"],
                                  inputs["m_wih"], inputs["m_whh"],
                                  inputs["m_bih"], inputs["m_bhh"], T)
        in_maps.append(m)
    res = run_bass_kernel_spmd(nc_a, in_maps, core_ids=list(range(8)),
                               trace=trace)
    def ctx_of(i):
        return res.results[i]["ctx"] / res.results[i]["den"]
    c_matrix = np.concatenate([ctx_of(i) for i in range(4)], 0)
    macro_ctx = np.concatenate([ctx_of(i) for i in range(4, 8)], 0)
    return c_matrix, macro_ctx, res


def run_post_launch(c_matrix, macro_ctx, inputs, trace=False):
    if "post" not in _progs:
        _progs["post"] = build_post_program()
    nc_b = _progs["post"]
    in_maps = [
        _post_core_inputs(c_matrix[c * BC:(c + 1) * BC],
                          macro_ctx[c * BC:(c + 1) * BC], inputs)
        for c in range(8)
    ]
    res = run_bass_kernel_spmd(nc_b, in_maps, core_ids=list(range(8)),
                               trace=trace)
    fb = float(np.asarray(inputs["final_b"]).reshape(-1)[0])
    out = np.concatenate(
        [res.results[i]["out"].reshape(BC) for i in range(8)], 0) + fb
    return out.astype(np.float32), res


def kernel(**inputs):
    c_matrix, macro_ctx, _ = run_lstm_launch(inputs)
    out, _ = run_post_launch(c_matrix, macro_ctx, inputs)
    return out



# revision 29
# speedup vs baseline: 2.3256x; 2.3256x over previous
"""DTML model kernel for 8 Trainium2 NeuronCores.

Two SPMD launches:
  A: one attention-LSTM per core. Cores 0-3 run the "stock" LSTM on batch
     quarters (B=128 each), cores 4-7 the "macro" LSTM on the same batch
     quarters. Identical program; per-core inputs pick weights/batch slice.
     Per core: input transform (tanh(x@W.T+b), transposed layout), 256-step
     LSTM recurrence, dot-attention over time. Output ctx [128, 256].
  B: post-phase (normalize + 32-token MHA + MLP + head), data-parallel over
     batch (64 per core), feature-on-partitions layout.

Launch A layouts (per core, B=128, T=256, D=128, H=256):
  gates [B part, 4H free], gate column order [f|i|g|o] (weights permuted on
  host). Recurrent matmul out = lhsT.T @ rhs: lhsT = data (stockinT_t, hT
  tiles, ones-row for the bias), rhs = weight rows streaming. h transposed
  each step on PE for the next step's lhsT.
"""

import sys

sys.path.insert(0, "/opt/trn_rl_repo")

from contextlib import ExitStack

import numpy as np

import concourse.bacc as bacc
import concourse.mybir as mybir
from concourse.masks import make_identity
from concourse.tile import TileContext
from concourse.bass_utils import run_bass_kernel_spmd

import ml_dtypes

FP = mybir.dt.float32
BF = mybir.dt.bfloat16
AX = mybir.AxisListType
OP = mybir.AluOpType
AF = mybir.ActivationFunctionType

BF_NP = ml_dtypes.bfloat16

B, S, D = 512, 256, 128
H = 256
NSTOCK = 32
NHEADS = 8
DH = H // NHEADS
BC = B // 8      # 64  batch per core in launch B
BL = B // 4      # 128 batch per core in launch A
NTOK = BC * NSTOCK  # 2048


# ----------------------------------------------------------------------------
# Launch A
# ----------------------------------------------------------------------------

def build_lstm_program(T=S):
    nc = bacc.Bacc("TRN2", target_bir_lowering=False, debug=False)

    x_d = nc.dram_tensor("x", [BL, T * D], FP, kind="ExternalInput")
    wtr_d = nc.dram_tensor("wtr", [D, D], BF, kind="ExternalInput")      # tr_w.T
    btr_d = nc.dram_tensor("btr", [D, 1], FP, kind="ExternalInput")
    wih_d = nc.dram_tensor("wih", [D, 4 * H], BF, kind="ExternalInput")  # wih[perm].T
    whh_d = nc.dram_tensor("whh", [H, 4 * H], BF, kind="ExternalInput")  # whh[perm].T
    bias_d = nc.dram_tensor("bias", [1, 4 * H], BF, kind="ExternalInput")
    ctx_d = nc.dram_tensor("ctx", [BL, H], FP, kind="ExternalOutput")
    den_d = nc.dram_tensor("den", [BL, 1], FP, kind="ExternalOutput")

    sinT_d = nc.dram_tensor("sinT_d", [D, T * BL], BF)
    hs_d = nc.dram_tensor("hs_d", [BL, T * H], FP)

    with TileContext(nc) as tc, ExitStack() as big:
        consts = big.enter_context(tc.tile_pool(name="consts", bufs=1))
        ident = consts.tile([128, 128], FP, name="ident")
        make_identity(nc, ident[:])
        wtr = consts.tile([D, D], BF, name="wtr")
        nc.sync.dma_start(out=wtr[:], in_=wtr_d.ap())
        btr = consts.tile([D, 1], FP, name="btr")
        nc.sync.dma_start(out=btr[:], in_=btr_d.ap())
        wih = consts.tile([D, 4 * H], BF, name="wih")
        nc.sync.dma_start(out=wih[:], in_=wih_d.ap())
        whh0 = consts.tile([128, 4 * H], BF, name="whh0")
        nc.sync.dma_start(out=whh0[:], in_=whh_d.ap()[0:128, :])
        whh1 = consts.tile([128, 4 * H], BF, name="whh1")
        nc.sync.dma_start(out=whh1[:], in_=whh_d.ap()[128:256, :])
        biasr = consts.tile([1, 4 * H], BF, name="biasr")
        nc.sync.dma_start(out=biasr[:], in_=bias_d.ap())
        ones1 = consts.tile([1, 128], BF, name="ones1")
        nc.gpsimd.memset(ones1[:], 1.0)

        # ---- pre-phase: sinT_t = tanh(wtr.T @ x_tT + btr), stored to DRAM
        PRE = 4
        with tc.tile_pool(name="pre_sb", bufs=3) as pre_sb, \
             tc.tile_pool(name="pre_ps", bufs=2, space="PSUM") as pre_ps, \
             tc.tile_pool(name="pre_o", bufs=3) as pre_o:
            for t0 in range(0, T, PRE):
                xc = pre_sb.tile([BL, PRE * D], FP, name="xc")
                nc.sync.dma_start(
                    out=xc[:], in_=x_d.ap()[:, t0 * D:(t0 + PRE) * D])
                xT_ps = pre_ps.tile([128, PRE * 128], FP, name="xT_ps")
                xT_sb = pre_sb.tile([128, PRE * 128], BF, name="xT_sb")
                tf_ps = pre_ps.tile([128, PRE * 128], FP, name="tf_ps")
                sc = pre_o.tile([128, PRE * 128], BF, name="sc")
                for j in range(PRE):
                    sl = slice(j * 128, (j + 1) * 128)
                    nc.tensor.transpose(xT_ps[:, sl], xc[:, sl], ident[:])
                    nc.vector.tensor_copy(xT_sb[:, sl], xT_ps[:, sl])
                    nc.tensor.matmul(tf_ps[:, sl], wtr[:], xT_sb[:, sl],
                                     start=True, stop=True)
                nc.scalar.activation(sc[:], tf_ps[:], AF.Tanh, bias=btr[:])
                nc.sync.dma_start(
                    out=sinT_d.ap()[:, t0 * BL:(t0 + PRE) * BL], in_=sc[:])

        # ---- recurrence
        state = big.enter_context(tc.tile_pool(name="state", bufs=1))
        hT = state.tile([128, 2 * BL], BF, name="hT")
        c = state.tile([BL, H], FP, name="c")

        rec_sin = big.enter_context(tc.tile_pool(name="rec_sin", bufs=4))
        rec_ps = big.enter_context(tc.tile_pool(name="rec_ps", bufs=2, space="PSUM"))
        rec_tr = big.enter_context(tc.tile_pool(name="rec_tr", bufs=2, space="PSUM"))
        rec_sb = big.enter_context(tc.tile_pool(name="rec_sb", bufs=3))
        rec_h = big.enter_context(tc.tile_pool(name="rec_h", bufs=3))

        h_prev = None
        for t in range(T):
            sin = rec_sin.tile([128, BL], BF, name="sin")
            nc.sync.dma_start(
                out=sin[:], in_=sinT_d.ap()[:, t * BL:(t + 1) * BL])
            gates = rec_ps.tile([BL, 4 * H], FP, name="gates")
            for bk in range(2):
                ns = slice(bk * 512, (bk + 1) * 512)
                nc.tensor.matmul(gates[:, ns], sin[:], wih[:, ns],
                                 start=True, stop=False)
                nc.tensor.matmul(gates[:, ns], ones1[:], biasr[:, ns],
                                 start=False, stop=(t == 0))
            if t > 0:
                trp = rec_tr.tile([128, 2 * BL], FP, name="trp")
                for k in range(2):
                    sl = slice(k * 128, (k + 1) * 128)
                    nc.tensor.transpose(trp[:, sl], h_prev[:, sl], ident[:])
                    nc.vector.tensor_copy(hT[:, sl], trp[:, sl])
                for bk in range(2):
                    ns = slice(bk * 512, (bk + 1) * 512)
                    nc.tensor.matmul(gates[:, ns], hT[:, 0:BL], whh0[:, ns],
                                     start=False, stop=False)
                    nc.tensor.matmul(gates[:, ns], hT[:, BL:2 * BL],
                                     whh1[:, ns], start=False, stop=True)
            # gate cols [f|i|g|o]
            sfi = rec_sb.tile([BL, 512], FP, name="sfi")
            nc.scalar.activation(sfi[:], gates[:, 0:512], AF.Sigmoid)
            tg = rec_sb.tile([BL, H], FP, name="tg")
            nc.scalar.activation(tg[:], gates[:, 512:768], AF.Tanh)
            so = rec_sb.tile([BL, H], FP, name="so")
            nc.scalar.activation(so[:], gates[:, 768:1024], AF.Sigmoid)
            if t == 0:
                nc.vector.tensor_tensor(c[:], sfi[:, 256:512], tg[:], OP.mult)
            else:
                m2 = rec_sb.tile([BL, H], FP, name="m2")
                nc.vector.tensor_tensor(m2[:], sfi[:, 0:256], c[:], OP.mult)
                m1 = rec_sb.tile([BL, H], FP, name="m1")
                nc.vector.tensor_tensor(m1[:], sfi[:, 256:512], tg[:], OP.mult)
                nc.vector.tensor_tensor(c[:], m1[:], m2[:], OP.add)
            tct = rec_sb.tile([BL, H], FP, name="tct")
            nc.scalar.activation(tct[:], c[:], AF.Tanh)
            h = rec_h.tile([BL, H], FP, name="h")
            nc.vector.tensor_tensor(h[:], so[:], tct[:], OP.mult)
            nc.sync.dma_start(out=hs_d.ap()[:, t * H:(t + 1) * H], in_=h[:])
            h_prev = h

        # ---- attention over time
        att = big.enter_context(tc.tile_pool(name="att", bufs=1))
        hlast = att.tile([BL, H], FP, name="hlast")
        nc.vector.tensor_copy(hlast[:], h_prev[:])
        score = att.tile([BL, T], FP, name="score")
        ACH = min(8, T)
        with tc.tile_pool(name="att_in", bufs=3) as att_in, \
             tc.tile_pool(name="att_junk", bufs=4) as att_junk:
            for t0 in range(0, T, ACH):
                hc = att_in.tile([BL, ACH * H], FP, name="hc")
                nc.sync.dma_start(
                    out=hc[:], in_=hs_d.ap()[:, t0 * H:(t0 + ACH) * H])
                for j in range(ACH):
                    jk = att_junk.tile([BL, H], FP, name="jk")
                    nc.vector.scalar_tensor_tensor(
                        jk[:], hc[:, j * H:(j + 1) * H], 0.0, hlast[:],
                        OP.bypass, OP.mult,
                        accum_out=score[:, t0 + j:t0 + j + 1])
            denom = att.tile([BL, 1], FP, name="denom")
            nc.vector.tensor_reduce(denom[:], score[:], AX.X, OP.add)
            nc.sync.dma_start(out=den_d.ap(), in_=denom[:])
            ctxA = att.tile([BL, H], FP, name="ctxA")
            nc.vector.memset(ctxA[:], 0.0)
            ctxB = att.tile([BL, H], FP, name="ctxB")
            nc.gpsimd.memset(ctxB[:], 0.0)
            for t0 in range(0, T, ACH):
                hc = att_in.tile([BL, ACH * H], FP, name="hc2")
                nc.sync.dma_start(
                    out=hc[:], in_=hs_d.ap()[:, t0 * H:(t0 + ACH) * H])
                for j in range(ACH):
                    t = t0 + j
                    acc = ctxA if j % 2 == 0 else ctxB
                    nc.vector.scalar_tensor_tensor(
                        acc[:], hc[:, j * H:(j + 1) * H],
                        score[:, t:t + 1], acc[:], OP.mult, OP.add)
            ctx = att.tile([BL, H], FP, name="ctx")
            nc.vector.tensor_tensor(ctx[:], ctxA[:], ctxB[:], OP.add)
            nc.sync.dma_start(out=ctx_d.ap(), in_=ctx[:])

    nc.finalize()
    return nc


_PERM = None


def _gate_perm():
    global _PERM
    if _PERM is None:
        i0 = np.arange(H)
        _PERM = np.concatenate([H + i0, i0, 2 * H + i0, 3 * H + i0])  # f,i,g,o
    return _PERM


def _lstm_core_inputs(x_np, tr_w, tr_b, wih, whh, bih, bhh, T):
    perm = _gate_perm()
    f32 = lambda a: np.ascontiguousarray(np.asarray(a, np.float32))
    bf16 = lambda a: np.ascontiguousarray(np.asarray(a, np.float32)).astype(BF_NP)
    return {
        "x": f32(x_np.reshape(BL, T * D)),
        "wtr": bf16(np.asarray(tr_w).T),
        "btr": f32(np.asarray(tr_b).reshape(D, 1)),
        "wih": bf16(np.asarray(wih)[perm].T),
        "whh": bf16(np.asarray(whh)[perm].T),
        "bias": bf16((np.asarray(bih) + np.asarray(bhh))[perm].reshape(1, 4 * H)),
    }


# ----------------------------------------------------------------------------
# Launch B
# ----------------------------------------------------------------------------

def build_post_program():
    nc = bacc.Bacc("TRN2", target_bir_lowering=False, debug=False)

    uT_d = nc.dram_tensor("uT", [H, BC], FP, kind="ExternalInput")   # ((cm-mean)/std).T
    vT_d = nc.dram_tensor("vT", [H, BC], FP, kind="ExternalInput")   # (mw*mc).T
    nwT_d = nc.dram_tensor("nwT", [H, NSTOCK], FP, kind="ExternalInput")
    nbT_d = nc.dram_tensor("nbT", [H, NSTOCK], FP, kind="ExternalInput")
    ipwT_d = nc.dram_tensor("ipwT", [H, 3 * H], BF, kind="ExternalInput")  # q-cols pre-scaled
    ipb_d = nc.dram_tensor("ipb", [32, 16], FP, kind="ExternalInput")      # [32d, q8|k8]
    opwT_d = nc.dram_tensor("opwT", [H, H], BF, kind="ExternalInput")
    opb_d = nc.dram_tensor("opb", [128, 2], FP, kind="ExternalInput")      # bo' per-ptile
    w1T_d = nc.dram_tensor("w1T", [H, 4 * H], BF, kind="ExternalInput")
    b1_d = nc.dram_tensor("b1", [128, 8], FP, kind="ExternalInput")
    w2T_d = nc.dram_tensor("w2T", [4 * H, H], BF, kind="ExternalInput")
    b2_d = nc.dram_tensor("b2", [128, 2], FP, kind="ExternalInput")
    fw_d = nc.dram_tensor("fw", [H, 1], FP, kind="ExternalInput")          # final_w/NSTOCK
    out_d = nc.dram_tensor("out", [BC, 1], FP, kind="ExternalOutput")

    NCH = 512

    with TileContext(nc) as tc, ExitStack() as big:
        P = big.enter_context(tc.tile_pool(name="main", bufs=1))
        ps = big.enter_context(tc.tile_pool(name="ps", bufs=2, space="PSUM"))

        def load(name, dram, shape):
            tl = P.tile(shape, FP, name=name)
            nc.sync.dma_start(out=tl[:], in_=dram.ap())
            return tl

        ipb = load("ipb", ipb_d, [32, 16])
        opb = load("opb", opb_d, [128, 2])
        b1 = load("b1", b1_d, [128, 8])
        b2 = load("b2", b2_d, [128, 2])
        fw = P.tile([128, 2], FP, name="fw")
        nc.sync.dma_start(
            out=fw[:].rearrange("p (k o) -> p k o", k=2),
            in_=fw_d.ap().rearrange("(k p) o -> p k o", p=128))
        # weights, stored as [128, ktiles*cols] blocks
        ipwT = P.tile([128, 2 * 768], BF, name="ipwT")
        nc.sync.dma_start(
            out=ipwT[:].rearrange("p (k n) -> p k n", k=2),
            in_=ipwT_d.ap().rearrange("(k p) n -> p k n", p=128))
        opwT = P.tile([128, 2 * 256], BF, name="opwT")
        nc.sync.dma_start(
            out=opwT[:].rearrange("p (k n) -> p k n", k=2),
            in_=opwT_d.ap().rearrange("(k p) n -> p k n", p=128))
        w1T = P.tile([128, 2 * 1024], BF, name="w1T")
        nc.sync.dma_start(
            out=w1T[:].rearrange("p (k n) -> p k n", k=2),
            in_=w1T_d.ap().rearrange("(k p) n -> p k n", p=128))
        w2T = P.tile([128, 8 * 256], BF, name="w2T")
        nc.sync.dma_start(
            out=w2T[:].rearrange("p (k n) -> p k n", k=8),
            in_=w2T_d.ap().rearrange("(k p) n -> p k n", p=128))

        uT2 = P.tile([128, 2 * BC], FP, name="uT2")
        nc.sync.dma_start(
            out=uT2[:].rearrange("p (k n) -> p k n", k=2),
            in_=uT_d.ap().rearrange("(k p) n -> p k n", p=128))
        vT2 = P.tile([128, 2 * BC], FP, name="vT2")
        nc.sync.dma_start(
            out=vT2[:].rearrange("p (k n) -> p k n", k=2),
            in_=vT_d.ap().rearrange("(k p) n -> p k n", p=128))
        nwT2 = P.tile([128, 2 * NSTOCK], FP, name="nwT2")
        nc.sync.dma_start(
            out=nwT2[:].rearrange("p (k n) -> p k n", k=2),
            in_=nwT_d.ap().rearrange("(k p) n -> p k n", p=128))
        nbT2 = P.tile([128, 2 * NSTOCK], FP, name="nbT2")
        nc.sync.dma_start(
            out=nbT2[:].rearrange("p (k n) -> p k n", k=2),
            in_=nbT_d.ap().rearrange("(k p) n -> p k n", p=128))

        # ---- mlT [128, 2*NTOK], col = b*32+s  (bf16 for PE; fp32 copy for DVE)
        mlT = P.tile([128, 2 * NTOK], BF, name="mlT")
        mlTf = P.tile([128, 2 * NTOK], FP, name="mlTf")
        tmp = P.tile([128, NTOK], FP, name="tmp")
        for k in range(2):
            msl = slice(k * NTOK, (k + 1) * NTOK)
            nc.vector.tensor_tensor(
                tmp[:].rearrange("p (b s) -> p b s", s=NSTOCK),
                nwT2[:, k * NSTOCK:(k + 1) * NSTOCK].unsqueeze(1)
                    .broadcast_to([128, BC, NSTOCK]),
                uT2[:, k * BC:(k + 1) * BC].unsqueeze(2)
                    .broadcast_to([128, BC, NSTOCK]),
                OP.mult)
            nc.vector.tensor_tensor(
                mlTf[:, msl].rearrange("p (b s) -> p b s", s=NSTOCK),
                nbT2[:, k * NSTOCK:(k + 1) * NSTOCK].unsqueeze(1)
                    .broadcast_to([128, BC, NSTOCK]),
                vT2[:, k * BC:(k + 1) * BC].unsqueeze(2)
                    .broadcast_to([128, BC, NSTOCK]),
                OP.add)
            nc.vector.tensor_tensor(mlTf[:, msl], mlTf[:, msl], tmp[:], OP.add)
            nc.scalar.copy(mlT[:, msl], mlTf[:, msl])

        # ---- MHA in b-chunks
        attnT = P.tile([128, 2 * NTOK], BF, name="attnT")
        ones32 = P.tile([32, 32], BF, name="ones32")
        nc.gpsimd.memset(ones32[:], 1.0)
        BCH = 8
        with tc.tile_pool(name="mha_sb", bufs=1) as mha_sb, \
             tc.tile_pool(name="mha_ps", bufs=2, space="PSUM") as mha_ps, \
             tc.tile_pool(name="den_ps", bufs=2, space="PSUM") as den_ps:
            for b0 in range(0, BC, BCH):
                # q32/k32 chunks [32 d-part, (hd, bw, s)]
                q32 = mha_sb.tile([32, NHEADS * BCH * 32], BF, name="q32")
                k32 = mha_sb.tile([32, NHEADS * BCH * 32], BF, name="k32")
                for hd in range(NHEADS):
                    for qk in range(2):
                        qp = mha_ps.tile([32, BCH * 32], FP, name="qp", tag="mps")
                        for k in range(2):
                            nc.tensor.matmul(
                                qp[:],
                                ipwT[:, k * 768 + qk * 256 + hd * 32:
                                     k * 768 + qk * 256 + (hd + 1) * 32],
                                mlT[:, k * NTOK + b0 * 32:
                                    k * NTOK + (b0 + BCH) * 32],
                                start=(k == 0), stop=(k == 1))
                        dst = q32 if qk == 0 else k32
                        nc.vector.tensor_scalar(
                            dst[:, hd * BCH * 32:(hd + 1) * BCH * 32], qp[:],
                            ipb[0:32, qk * 8 + hd:qk * 8 + hd + 1], None,
                            OP.add)
                # v_tok chunk [32, BCH*256], col = (b-b0)*256 + hd*32 + d
                v_tok = mha_sb.tile([32, BCH * 256], BF, name="v_tok")
                for bi in range(0, BCH, 2):
                    vp = mha_ps.tile([32, 512], FP, name="vp", tag="mps")
                    for bj in range(2):
                        b = b0 + bi + bj
                        for k in range(2):
                            nc.tensor.matmul(
                                vp[:, bj * 256:(bj + 1) * 256],
                                mlT[:, k * NTOK + b * 32:k * NTOK + (b + 1) * 32],
                                ipwT[:, k * 768 + 512:k * 768 + 768],
                                start=(k == 0), stop=(k == 1))
                    nc.vector.tensor_copy(
                        v_tok[:, bi * 256:(bi + 2) * 256], vp[:])
                # scoresT -> exp, esc chunk [32, BCH*256], col=(b-b0)*256+hd*32+s
                esc = mha_sb.tile([32, BCH * 256], BF, name="esc")
                for bi in range(0, BCH, 2):
                    sp = mha_ps.tile([32, 512], FP, name="sp", tag="mps")
                    for bj in range(2):
                        bw = bi + bj
                        for hd in range(NHEADS):
                            co = hd * BCH * 32 + bw * 32
                            nc.tensor.matmul(
                                sp[0:32, bj * 256 + hd * 32:bj * 256 + (hd + 1) * 32],
                                k32[0:32, co:co + 32],
                                q32[0:32, co:co + 32],
                                start=True, stop=True)
                    nc.scalar.activation(
                        esc[:, bi * 256:(bi + 2) * 256], sp[:], AF.Exp)
                # denominator: all-ones [32,32] matmul replicates the
                # partition-sum of esc onto all 32 partitions in one shot
                recip = mha_sb.tile([32, BCH * 256], FP, name="recip")
                for j in range(0, BCH * 256, 512):
                    rrep = den_ps.tile([32, 512], FP, name="rrep", tag="dps")
                    nc.tensor.matmul(rrep[:], ones32[:], esc[:, j:j + 512],
                                     start=True, stop=True)
                    nc.vector.reciprocal(recip[:, j:j + 512], rrep[:])
                nc.vector.tensor_tensor(esc[:], esc[:], recip[:], OP.mult)
                # AV: attnT chunk, 2 psum tiles per 4 b's
                for bi in range(0, BCH, 4):
                    for hf in range(2):
                        ap_ps = mha_ps.tile([128, 128], FP, name="ap_ps")
                        for bj in range(4):
                            b = b0 + bi + bj
                            for hq in range(4):
                                hd = hf * 4 + hq
                                col = (bi + bj) * 256 + hd * 32
                                nc.tensor.matmul(
                                    ap_ps[hq * 32:(hq + 1) * 32,
                                          bj * 32:(bj + 1) * 32],
                                    v_tok[0:32, col:col + 32],
                                    esc[0:32, col:col + 32],
                                    start=True, stop=True,
                                    tile_position=(0, hq * 32))
                        nc.vector.tensor_copy(
                            attnT[:, hf * NTOK + (b0 + bi) * 32:
                                  hf * NTOK + (b0 + bi + 4) * 32], ap_ps[:])

        # ---- att_outT + residual -> h1T (bf16 + fp32 copy for later residual)
        h1T = P.tile([128, 2 * NTOK], BF, name="h1T")
        h1Tf = P.tile([128, 2 * NTOK], FP, name="h1Tf")
        for m in range(2):
            for n0 in range(0, NTOK, NCH):
                sl = slice(m * NTOK + n0, m * NTOK + n0 + NCH)
                op_ps = ps.tile([128, NCH], FP, name="op_ps", tag="mmps")
                for k in range(2):
                    nc.tensor.matmul(
                        op_ps[:],
                        opwT[:, k * 256 + m * 128:k * 256 + (m + 1) * 128],
                        attnT[:, k * NTOK + n0:k * NTOK + n0 + NCH],
                        start=(k == 0), stop=(k == 1))
                nc.vector.scalar_tensor_tensor(
                    h1Tf[:, sl], op_ps[:], opb[:, m:m + 1], mlTf[:, sl],
                    OP.add, OP.add)
                nc.scalar.copy(h1T[:, sl], h1Tf[:, sl])

        # ---- MLP fused over n-chunks; outT = tanh(h1T + mlp)
        outT = P.tile([128, 2 * NTOK], FP, name="outT")
        with tc.tile_pool(name="mid_sb", bufs=2) as mid_sb:
            for n0 in range(0, NTOK, NCH):
                mid = mid_sb.tile([128, 8 * NCH], BF, name="mid")
                for m in range(8):
                    mp = ps.tile([128, NCH], FP, name="mp", tag="mmps")
                    for k in range(2):
                        nc.tensor.matmul(
                            mp[:],
                            w1T[:, k * 1024 + m * 128:k * 1024 + (m + 1) * 128],
                            h1T[:, k * NTOK + n0:k * NTOK + n0 + NCH],
                            start=(k == 0), stop=(k == 1))
                    nc.vector.tensor_scalar(
                        mid[:, m * NCH:(m + 1) * NCH], mp[:],
                        b1[:, m:m + 1], 0.0, OP.add, op1=OP.max)
                for m in range(2):
                    op2 = ps.tile([128, NCH], FP, name="op2", tag="mmps")
                    for k in range(8):
                        nc.tensor.matmul(
                            op2[:],
                            w2T[:, k * 256 + m * 128:k * 256 + (m + 1) * 128],
                            mid[:, k * NCH:(k + 1) * NCH],
                            start=(k == 0), stop=(k == 7))
                    pre = mid_sb.tile([128, NCH], FP, name="pre")
                    nc.vector.scalar_tensor_tensor(
                        pre[:], op2[:], b2[:, m:m + 1],
                        h1Tf[:, m * NTOK + n0:m * NTOK + n0 + NCH],
                        OP.add, OP.add)
                    nc.scalar.activation(
                        outT[:, m * NTOK + n0:m * NTOK + n0 + NCH], pre[:],
                        AF.Tanh)

        # ---- pool over s, final head
        pooledT = P.tile([128, 2 * BC], FP, name="pooledT")
        for k in range(2):
            nc.vector.tensor_reduce(
                pooledT[:, k * BC:(k + 1) * BC],
                outT[:, k * NTOK:(k + 1) * NTOK].rearrange(
                    "p (b s) -> p b s", s=NSTOCK),
                AX.X, OP.add)
        fin_ps = ps.tile([BC, 1], FP, name="fin_ps", tag="mmps")
        for k in range(2):
            nc.tensor.matmul(fin_ps[:], pooledT[:, k * BC:(k + 1) * BC],
                             fw[:, k:k + 1],
                             start=(k == 0), stop=(k == 1))
        fin = P.tile([BC, 1], FP, name="fin")
        nc.vector.tensor_copy(fin[:], fin_ps[:])
        nc.sync.dma_start(out=out_d.ap(), in_=fin[:])

    nc.finalize()
    return nc


def _post_core_inputs(cm_b, mc_b, inputs):
    f32 = lambda a: np.ascontiguousarray(np.asarray(a, np.float32))
    mw = float(np.asarray(inputs["macro_weight"]).reshape(-1)[0])
    mean = cm_b.mean(1, keepdims=True)
    std = cm_b.std(1, keepdims=True, ddof=1) + 1e-8
    uT = ((cm_b - mean) / std).T
    vT = (mc_b * mw).T
    ipw = np.asarray(inputs["in_proj_w"], np.float32)
    ipb = np.asarray(inputs["in_proj_b"], np.float32)
    opw = np.asarray(inputs["out_proj_w"], np.float32)
    opb = np.asarray(inputs["out_proj_b"], np.float32)
    qsc = 1.0 / np.sqrt(DH)
    ipwT = ipw.T.copy()
    ipwT[:, 0:H] *= qsc
    ipb_eff = ipb.copy()
    ipb_eff[0:H] *= qsc
    opb_eff = opb + ipb[2 * H:] @ opw.T
    bf16 = lambda a: np.ascontiguousarray(np.asarray(a, np.float32)).astype(BF_NP)
    return {
        "uT": f32(uT),
        "vT": f32(vT),
        "nwT": f32(np.asarray(inputs["norm_weight"]).T),
        "nbT": f32(np.asarray(inputs["norm_bias"]).T),
        "ipwT": bf16(ipwT),
        "ipb": f32(ipb_eff[0:2 * H].reshape(2, 8, 32).transpose(2, 0, 1).reshape(32, 16)),
        "opwT": bf16(opw.T),
        "opb": f32(opb_eff.reshape(2, 128).T),
        "w1T": bf16(np.asarray(inputs["mlp_w1"]).T),
        "b1": f32(np.asarray(inputs["mlp_b1"]).reshape(8, 128).T),
        "w2T": bf16(np.asarray(inputs["mlp_w2"]).T),
        "b2": f32(np.asarray(inputs["mlp_b2"]).reshape(2, 128).T),
        "fw": f32((np.asarray(inputs["final_w"]).reshape(H) / NSTOCK).reshape(H, 1)),
    }


# ----------------------------------------------------------------------------
# host orchestration
# ----------------------------------------------------------------------------

_progs = {}


def run_lstm_launch(inputs, T=S, trace=False):
    if ("lstm", T) not in _progs:
        _progs[("lstm", T)] = build_lstm_program(T)
    nc_a = _progs[("lstm", T)]
    x = np.asarray(inputs["x"], np.float32)
    in_maps = []
    for core in range(8):
        q = core % 4
        xb = x[q * BL:(q + 1) * BL]
        if core < 4:
            m = _lstm_core_inputs(xb, inputs["stock_tr_w"], inputs["stock_tr_b"],
                                  inputs["s_wih"], inputs["s_whh"],
                                  inputs["s_bih"], inputs["s_bhh"], T)
        else:
            m = _lstm_core_inputs(xb, inputs["macro_tr_w"], inputs["macro_tr_b"],
                                  inputs["m_wih"], inputs["m_whh"],
                                  inputs["m_bih"], inputs["m_bhh"], T)
        in_maps.append(m)
    res = run_bass_kernel_spmd(nc_a, in_maps, core_ids=list(range(8)),
                               trace=trace)
    def ctx_of(i):
        return res.results[i]["ctx"] / res.results[i]["den"]
    c_matrix = np.concatenate([ctx_of(i) for i in range(4)], 0)
    macro_ctx = np.concatenate([ctx_of(i) for i in range(4, 8)], 0)
    return c_matrix, macro_ctx, res


def run_post_launch(c_matrix, macro_ctx, inputs, trace=False):
    if "post" not in _progs:
        _progs["post"] = build_post_program()
    nc_b = _progs["post"]
    in_maps = [
        _post_core_inputs(c_matrix[c * BC:(c + 1) * BC],
                          macro_ctx[c * BC:(c + 1) * BC], inputs)
        for c in range(8)
    ]
    res = run_bass_kernel_spmd(nc_b, in_maps, core_ids=list(range(8)),
                               trace=trace)
    fb = float(np.asarray(inputs["final_b"]).reshape(-1)[0])
    out = np.concatenate(
        [res.results[i]["out"].reshape(BC) for i in range(8)], 0) + fb
    return out.astype(np.float32), res


def kernel(**inputs):
    c_matrix, macro_ctx, _ = run_lstm_launch(inputs)
    out, _ = run_post_launch(c_matrix, macro_ctx, inputs)
    return out



# revision 38
# speedup vs baseline: 2.4558x; 1.0560x over previous
"""DTML model kernel for 8 Trainium2 NeuronCores.

Two SPMD launches:
  A: one attention-LSTM per core. Cores 0-3 run the "stock" LSTM on batch
     quarters (B=128 each), cores 4-7 the "macro" LSTM on the same batch
     quarters. Identical program; per-core inputs pick weights/batch slice.
     Per core: input transform (tanh(x@W.T+b), transposed layout), 256-step
     LSTM recurrence, dot-attention over time. Output ctx [128, 256].
  B: post-phase (normalize + 32-token MHA + MLP + head), data-parallel over
     batch (64 per core), feature-on-partitions layout.

Launch A layouts (per core, B=128, T=256, D=128, H=256):
  gates [B part, 4H free], gate column order [f|i|g|o] (weights permuted on
  host). Recurrent matmul out = lhsT.T @ rhs: lhsT = data (stockinT_t, hT
  tiles, ones-row for the bias), rhs = weight rows streaming. h transposed
  each step on PE for the next step's lhsT.
"""

import sys

sys.path.insert(0, "/opt/trn_rl_repo")

from contextlib import ExitStack

import numpy as np

import concourse.bacc as bacc
import concourse.mybir as mybir
from concourse.masks import make_identity
from concourse.tile import TileContext
from concourse.bass_utils import run_bass_kernel_spmd

import ml_dtypes

FP = mybir.dt.float32
BF = mybir.dt.bfloat16
AX = mybir.AxisListType
OP = mybir.AluOpType
AF = mybir.ActivationFunctionType

BF_NP = ml_dtypes.bfloat16

B, S, D = 512, 256, 128
H = 256
NSTOCK = 32
NHEADS = 8
DH = H // NHEADS
BC = B // 8      # 64  batch per core in launch B
BL = B // 4      # 128 batch per core in launch A
NTOK = BC * NSTOCK  # 2048


# ----------------------------------------------------------------------------
# Launch A
# ----------------------------------------------------------------------------

def build_lstm_program(T=S):
    nc = bacc.Bacc("TRN2", target_bir_lowering=False, debug=False)

    x_d = nc.dram_tensor("x", [BL, T * D], FP, kind="ExternalInput")
    wtr_d = nc.dram_tensor("wtr", [D, D], BF, kind="ExternalInput")      # tr_w.T
    btr_d = nc.dram_tensor("btr", [D, 1], FP, kind="ExternalInput")
    wih_d = nc.dram_tensor("wih", [D, 4 * H], BF, kind="ExternalInput")  # wih[perm].T
    whh_d = nc.dram_tensor("whh", [H, 4 * H], BF, kind="ExternalInput")  # whh[perm].T
    bias_d = nc.dram_tensor("bias", [1, 4 * H], BF, kind="ExternalInput")
    ctx_d = nc.dram_tensor("ctx", [BL, H], FP, kind="ExternalOutput")
    den_d = nc.dram_tensor("den", [BL, 1], FP, kind="ExternalOutput")

    PRE = 4       # timesteps per pre-phase chunk
    LOOK = 8      # pre-phase lookahead (steps)

    with TileContext(nc) as tc, ExitStack() as big:
        consts = big.enter_context(tc.tile_pool(name="consts", bufs=1))
        ident = consts.tile([128, 128], FP, name="ident")
        make_identity(nc, ident[:])
        identB = consts.tile([128, 128], BF, name="identB")
        make_identity(nc, identB[:])
        wtr = consts.tile([D, D], BF, name="wtr")
        nc.sync.dma_start(out=wtr[:], in_=wtr_d.ap())
        btr = consts.tile([D, 1], FP, name="btr")
        nc.sync.dma_start(out=btr[:], in_=btr_d.ap())
        wih = consts.tile([D, 4 * H], BF, name="wih")
        nc.sync.dma_start(out=wih[:], in_=wih_d.ap())
        whh0 = consts.tile([128, 4 * H], BF, name="whh0")
        nc.sync.dma_start(out=whh0[:], in_=whh_d.ap()[0:128, :])
        whh1 = consts.tile([128, 4 * H], BF, name="whh1")
        nc.sync.dma_start(out=whh1[:], in_=whh_d.ap()[128:256, :])
        biasr = consts.tile([1, 4 * H], BF, name="biasr")
        nc.sync.dma_start(out=biasr[:], in_=bias_d.ap())
        ones1 = consts.tile([1, 128], BF, name="ones1")
        nc.gpsimd.memset(ones1[:], 1.0)

        # all hidden states stay resident in SBUF (bf16: 128KB/partition)
        hs = big.enter_context(tc.tile_pool(name="hs", bufs=1)) \
            .tile([BL, T * H], BF, name="hs")

        state = big.enter_context(tc.tile_pool(name="state", bufs=1))
        # cg = [c | g~] so the c-update runs as one 512-wide mult + one add
        cg = state.tile([BL, 512], FP, name="cg")
        hT = state.tile([128, 2 * BL], BF, name="hT")

        pre_x = big.enter_context(tc.tile_pool(name="pre_x", bufs=3))
        pre_ps = big.enter_context(tc.tile_pool(name="pre_ps", bufs=1, space="PSUM"))
        pre_tf = big.enter_context(tc.tile_pool(name="pre_tf", bufs=1, space="PSUM"))
        pre_sc = big.enter_context(tc.tile_pool(name="pre_sc", bufs=4))

        sc_tiles = {}

        def emit_pre_chunk(ci):
            t0 = ci * PRE
            xc = pre_x.tile([BL, PRE * D], FP, name="xc")
            nc.sync.dma_start(out=xc[:], in_=x_d.ap()[:, t0 * D:(t0 + PRE) * D])
            xT_ps = pre_ps.tile([128, PRE * 128], FP, name="xT_ps")
            xT_sb = pre_x.tile([128, PRE * 128], BF, name="xT_sb")
            for j in range(PRE):
                sl = slice(j * 128, (j + 1) * 128)
                nc.tensor.transpose(xT_ps[:, sl], xc[:, sl], ident[:])
                nc.vector.tensor_copy(xT_sb[:, sl], xT_ps[:, sl])
            tf_ps = pre_tf.tile([128, PRE * 128], FP, name="tf_ps")
            nc.tensor.matmul(tf_ps[:], wtr[:], xT_sb[:], start=True, stop=True)
            sc = pre_sc.tile([128, PRE * BL], BF, name="sc")
            nc.scalar.activation(sc[:], tf_ps[:], AF.Tanh, bias=btr[:])
            sc_tiles[ci] = sc

        for ci in range((LOOK // PRE) + 1):
            emit_pre_chunk(ci)

        rec_ps = big.enter_context(tc.tile_pool(name="rec_ps", bufs=2, space="PSUM"))
        rec_tr = big.enter_context(tc.tile_pool(name="rec_tr", bufs=2, space="PSUM"))
        rec_sb = big.enter_context(tc.tile_pool(name="rec_sb", bufs=3))

        # gate cols [f|i|o|g] (host permutes weights): one sigmoid covers f,i,o
        for t in range(T):
            if t % PRE == 0:
                ci = (t + LOOK) // PRE + 1
                if ci * PRE < T and ci not in sc_tiles:
                    emit_pre_chunk(ci)
            sin = sc_tiles[t // PRE][:, (t % PRE) * BL:(t % PRE + 1) * BL]
            gates = rec_ps.tile([BL, 4 * H], FP, name="gates")
            for bk in range(2):
                ns = slice(bk * 512, (bk + 1) * 512)
                nc.tensor.matmul(gates[:, ns], sin, wih[:, ns],
                                 start=True, stop=False)
                nc.tensor.matmul(gates[:, ns], ones1[:], biasr[:, ns],
                                 start=False, stop=(t == 0))
            if t > 0:
                trp = rec_tr.tile([128, 2 * BL], BF, name="trp")
                hprev = hs[:, (t - 1) * H:t * H]
                for k in range(2):
                    sl = slice(k * 128, (k + 1) * 128)
                    nc.tensor.transpose(trp[:, sl],
                                        hprev[:, k * 128:(k + 1) * 128],
                                        identB[:])
                    nc.vector.tensor_copy(hT[:, sl], trp[:, sl])
                for kt in range(2):
                    whh = whh0 if kt == 0 else whh1
                    for bk in range(2):
                        ns = slice(bk * 512, (bk + 1) * 512)
                        nc.tensor.matmul(gates[:, ns],
                                         hT[:, kt * BL:(kt + 1) * BL],
                                         whh[:, ns],
                                         start=False, stop=(kt == 1))
            sfio = rec_sb.tile([BL, 768], FP, name="sfio")
            nc.scalar.activation(sfio[:], gates[:, 0:768], AF.Sigmoid)
            nc.scalar.activation(cg[:, 256:512], gates[:, 768:1024], AF.Tanh)
            if t == 0:
                nc.vector.tensor_tensor(cg[:, 0:256], sfio[:, 256:512],
                                        cg[:, 256:512], OP.mult)
            else:
                m = rec_sb.tile([BL, 512], FP, name="m")
                nc.vector.tensor_tensor(m[:], sfio[:, 0:512], cg[:], OP.mult)
                nc.vector.tensor_tensor(cg[:, 0:256], m[:, 0:256],
                                        m[:, 256:512], OP.add)
            tct = rec_sb.tile([BL, H], FP, name="tct")
            nc.scalar.activation(tct[:], cg[:, 0:256], AF.Tanh)
            nc.vector.tensor_tensor(hs[:, t * H:(t + 1) * H], sfio[:, 512:768],
                                    tct[:], OP.mult)

        # ---- attention over time (hs resident in SBUF; DVE + Pool split)
        att = big.enter_context(tc.tile_pool(name="att", bufs=1))
        hlast = hs[:, (T - 1) * H:T * H]
        score = att.tile([BL, T], FP, name="score")
        with tc.tile_pool(name="att_junk", bufs=4) as att_junk:
            for t in range(T):
                jk = att_junk.tile([BL, H], BF, name="jk")
                nc.vector.scalar_tensor_tensor(
                    jk[:], hs[:, t * H:(t + 1) * H], 0.0, hlast,
                    OP.bypass, OP.mult, accum_out=score[:, t:t + 1])
            accs = []
            for nm, eng in (("cA", nc.vector), ("cB", nc.vector),
                            ("cC", nc.gpsimd), ("cD", nc.gpsimd)):
                a = att.tile([BL, H], FP, name=nm)
                eng.memset(a[:], 0.0)
                accs.append(a)
            # DVE stt for most steps; Pool takes 1 in 4 via scale+add pairs
            for t in range(T):
                if t % 4 == 3:
                    acc = accs[2 + (t // 4) % 2]
                    pm = att_junk.tile([BL, H], FP, name="pm")
                    nc.gpsimd.tensor_scalar(
                        pm[:], hs[:, t * H:(t + 1) * H],
                        score[:, t:t + 1], None, OP.mult)
                    nc.gpsimd.tensor_tensor(acc[:], acc[:], pm[:], OP.add)
                else:
                    acc = accs[(t // 4) % 2]
                    nc.vector.scalar_tensor_tensor(
                        acc[:], hs[:, t * H:(t + 1) * H],
                        score[:, t:t + 1], acc[:], OP.mult, OP.add)
            denom = att.tile([BL, 1], FP, name="denom")
            nc.vector.tensor_reduce(denom[:], score[:], AX.X, OP.add)
            nc.sync.dma_start(out=den_d.ap(), in_=denom[:])
            nc.vector.tensor_tensor(accs[0][:], accs[0][:], accs[1][:], OP.add)
            nc.gpsimd.tensor_tensor(accs[2][:], accs[2][:], accs[3][:], OP.add)
            ctx = att.tile([BL, H], FP, name="ctx")
            nc.vector.tensor_tensor(ctx[:], accs[0][:], accs[2][:], OP.add)
            nc.sync.dma_start(out=ctx_d.ap(), in_=ctx[:])

    nc.finalize()
    return nc


_PERM = None


def _gate_perm():
    global _PERM
    if _PERM is None:
        i0 = np.arange(H)
        _PERM = np.concatenate([H + i0, i0, 3 * H + i0, 2 * H + i0])  # f,i,o,g
    return _PERM


def _lstm_core_inputs(x_np, tr_w, tr_b, wih, whh, bih, bhh, T):
    perm = _gate_perm()
    f32 = lambda a: np.ascontiguousarray(np.asarray(a, np.float32))
    bf16 = lambda a: np.ascontiguousarray(np.asarray(a, np.float32)).astype(BF_NP)
    return {
        "x": f32(x_np.reshape(BL, T * D)),
        "wtr": bf16(np.asarray(tr_w).T),
        "btr": f32(np.asarray(tr_b).reshape(D, 1)),
        "wih": bf16(np.asarray(wih)[perm].T),
        "whh": bf16(np.asarray(whh)[perm].T),
        "bias": bf16((np.asarray(bih) + np.asarray(bhh))[perm].reshape(1, 4 * H)),
    }


# ----------------------------------------------------------------------------
# Launch B
# ----------------------------------------------------------------------------

def build_post_program():
    nc = bacc.Bacc("TRN2", target_bir_lowering=False, debug=False)

    uT_d = nc.dram_tensor("uT", [H, BC], FP, kind="ExternalInput")   # ((cm-mean)/std).T
    vT_d = nc.dram_tensor("vT", [H, BC], FP, kind="ExternalInput")   # (mw*mc).T
    nwT_d = nc.dram_tensor("nwT", [H, NSTOCK], FP, kind="ExternalInput")
    nbT_d = nc.dram_tensor("nbT", [H, NSTOCK], FP, kind="ExternalInput")
    ipwT_d = nc.dram_tensor("ipwT", [H, 3 * H], BF, kind="ExternalInput")  # q-cols pre-scaled
    ipb_d = nc.dram_tensor("ipb", [32, 16], FP, kind="ExternalInput")      # [32d, q8|k8]
    opwT_d = nc.dram_tensor("opwT", [H, H], BF, kind="ExternalInput")
    opb_d = nc.dram_tensor("opb", [128, 2], FP, kind="ExternalInput")      # bo' per-ptile
    w1T_d = nc.dram_tensor("w1T", [H, 4 * H], BF, kind="ExternalInput")
    b1_d = nc.dram_tensor("b1", [128, 8], FP, kind="ExternalInput")
    w2T_d = nc.dram_tensor("w2T", [4 * H, H], BF, kind="ExternalInput")
    b2_d = nc.dram_tensor("b2", [128, 2], FP, kind="ExternalInput")
    fw_d = nc.dram_tensor("fw", [H, 1], FP, kind="ExternalInput")          # final_w/NSTOCK
    out_d = nc.dram_tensor("out", [BC, 1], FP, kind="ExternalOutput")

    NCH = 512

    with TileContext(nc) as tc, ExitStack() as big:
        P = big.enter_context(tc.tile_pool(name="main", bufs=1))
        ps = big.enter_context(tc.tile_pool(name="ps", bufs=2, space="PSUM"))

        def load(name, dram, shape):
            tl = P.tile(shape, FP, name=name)
            nc.sync.dma_start(out=tl[:], in_=dram.ap())
            return tl

        ipb = load("ipb", ipb_d, [32, 16])
        opb = load("opb", opb_d, [128, 2])
        b1 = load("b1", b1_d, [128, 8])
        b2 = load("b2", b2_d, [128, 2])
        fw = P.tile([128, 2], FP, name="fw")
        nc.sync.dma_start(
            out=fw[:].rearrange("p (k o) -> p k o", k=2),
            in_=fw_d.ap().rearrange("(k p) o -> p k o", p=128))
        # weights, stored as [128, ktiles*cols] blocks
        ipwT = P.tile([128, 2 * 768], BF, name="ipwT")
        nc.sync.dma_start(
            out=ipwT[:].rearrange("p (k n) -> p k n", k=2),
            in_=ipwT_d.ap().rearrange("(k p) n -> p k n", p=128))
        opwT = P.tile([128, 2 * 256], BF, name="opwT")
        nc.sync.dma_start(
            out=opwT[:].rearrange("p (k n) -> p k n", k=2),
            in_=opwT_d.ap().rearrange("(k p) n -> p k n", p=128))
        w1T = P.tile([128, 2 * 1024], BF, name="w1T")
        nc.sync.dma_start(
            out=w1T[:].rearrange("p (k n) -> p k n", k=2),
            in_=w1T_d.ap().rearrange("(k p) n -> p k n", p=128))
        w2T = P.tile([128, 8 * 256], BF, name="w2T")
        nc.sync.dma_start(
            out=w2T[:].rearrange("p (k n) -> p k n", k=8),
            in_=w2T_d.ap().rearrange("(k p) n -> p k n", p=128))

        uT2 = P.tile([128, 2 * BC], FP, name="uT2")
        nc.sync.dma_start(
            out=uT2[:].rearrange("p (k n) -> p k n", k=2),
            in_=uT_d.ap().rearrange("(k p) n -> p k n", p=128))
        vT2 = P.tile([128, 2 * BC], FP, name="vT2")
        nc.sync.dma_start(
            out=vT2[:].rearrange("p (k n) -> p k n", k=2),
            in_=vT_d.ap().rearrange("(k p) n -> p k n", p=128))
        nwT2 = P.tile([128, 2 * NSTOCK], FP, name="nwT2")
        nc.sync.dma_start(
            out=nwT2[:].rearrange("p (k n) -> p k n", k=2),
            in_=nwT_d.ap().rearrange("(k p) n -> p k n", p=128))
        nbT2 = P.tile([128, 2 * NSTOCK], FP, name="nbT2")
        nc.sync.dma_start(
            out=nbT2[:].rearrange("p (k n) -> p k n", k=2),
            in_=nbT_d.ap().rearrange("(k p) n -> p k n", p=128))

        # ---- mlT [128, 2*NTOK], col = b*32+s  (bf16 for PE; fp32 copy for DVE)
        mlT = P.tile([128, 2 * NTOK], BF, name="mlT")
        mlTf = P.tile([128, 2 * NTOK], FP, name="mlTf")
        tmp = P.tile([128, NTOK], FP, name="tmp")
        for k in range(2):
            msl = slice(k * NTOK, (k + 1) * NTOK)
            nc.vector.tensor_tensor(
                tmp[:].rearrange("p (b s) -> p b s", s=NSTOCK),
                nwT2[:, k * NSTOCK:(k + 1) * NSTOCK].unsqueeze(1)
                    .broadcast_to([128, BC, NSTOCK]),
                uT2[:, k * BC:(k + 1) * BC].unsqueeze(2)
                    .broadcast_to([128, BC, NSTOCK]),
                OP.mult)
            nc.vector.tensor_tensor(
                mlTf[:, msl].rearrange("p (b s) -> p b s", s=NSTOCK),
                nbT2[:, k * NSTOCK:(k + 1) * NSTOCK].unsqueeze(1)
                    .broadcast_to([128, BC, NSTOCK]),
                vT2[:, k * BC:(k + 1) * BC].unsqueeze(2)
                    .broadcast_to([128, BC, NSTOCK]),
                OP.add)
            nc.vector.tensor_tensor(mlTf[:, msl], mlTf[:, msl], tmp[:], OP.add)
            nc.scalar.copy(mlT[:, msl], mlTf[:, msl])

        # ---- MHA in b-chunks
        attnT = P.tile([128, 2 * NTOK], BF, name="attnT")
        ones32 = P.tile([32, 32], BF, name="ones32")
        nc.gpsimd.memset(ones32[:], 1.0)
        BCH = 8
        with tc.tile_pool(name="mha_sb", bufs=1) as mha_sb, \
             tc.tile_pool(name="mha_ps", bufs=2, space="PSUM") as mha_ps, \
             tc.tile_pool(name="den_ps", bufs=2, space="PSUM") as den_ps:
            for b0 in range(0, BC, BCH):
                # q32/k32 chunks [32 d-part, (hd, bw, s)]
                q32 = mha_sb.tile([32, NHEADS * BCH * 32], BF, name="q32")
                k32 = mha_sb.tile([32, NHEADS * BCH * 32], BF, name="k32")
                for hd in range(NHEADS):
                    for qk in range(2):
                        qp = mha_ps.tile([32, BCH * 32], FP, name="qp", tag="mps")
                        for k in range(2):
                            nc.tensor.matmul(
                                qp[:],
                                ipwT[:, k * 768 + qk * 256 + hd * 32:
                                     k * 768 + qk * 256 + (hd + 1) * 32],
                                mlT[:, k * NTOK + b0 * 32:
                                    k * NTOK + (b0 + BCH) * 32],
                                start=(k == 0), stop=(k == 1))
                        dst = q32 if qk == 0 else k32
                        nc.vector.tensor_scalar(
                            dst[:, hd * BCH * 32:(hd + 1) * BCH * 32], qp[:],
                            ipb[0:32, qk * 8 + hd:qk * 8 + hd + 1], None,
                            OP.add)
                # v_tok chunk [32, BCH*256], col = (b-b0)*256 + hd*32 + d
                v_tok = mha_sb.tile([32, BCH * 256], BF, name="v_tok")
                for bi in range(0, BCH, 2):
                    vp = mha_ps.tile([32, 512], FP, name="vp", tag="mps")
                    for bj in range(2):
                        b = b0 + bi + bj
                        for k in range(2):
                            nc.tensor.matmul(
                                vp[:, bj * 256:(bj + 1) * 256],
                                mlT[:, k * NTOK + b * 32:k * NTOK + (b + 1) * 32],
                                ipwT[:, k * 768 + 512:k * 768 + 768],
                                start=(k == 0), stop=(k == 1))
                    nc.scalar.copy(
                        v_tok[:, bi * 256:(bi + 2) * 256], vp[:])
                # scoresT -> exp, esc chunk [32, BCH*256], col=(b-b0)*256+hd*32+s
                esc = mha_sb.tile([32, BCH * 256], BF, name="esc")
                for bi in range(0, BCH, 2):
                    sp = mha_ps.tile([32, 512], FP, name="sp", tag="mps")
                    for bj in range(2):
                        bw = bi + bj
                        for hd in range(NHEADS):
                            co = hd * BCH * 32 + bw * 32
                            nc.tensor.matmul(
                                sp[0:32, bj * 256 + hd * 32:bj * 256 + (hd + 1) * 32],
                                k32[0:32, co:co + 32],
                                q32[0:32, co:co + 32],
                                start=True, stop=True)
                    nc.scalar.activation(
                        esc[:, bi * 256:(bi + 2) * 256], sp[:], AF.Exp)
                # denominator: all-ones [32,32] matmul replicates the
                # partition-sum of esc onto all 32 partitions in one shot;
                # reciprocal as exp(-ln(x)) on the otherwise-idle ACT engine
                lnr = mha_sb.tile([32, BCH * 256], FP, name="lnr")
                for j in range(0, BCH * 256, 512):
                    rrep = den_ps.tile([32, 512], FP, name="rrep", tag="dps")
                    nc.tensor.matmul(rrep[:], ones32[:], esc[:, j:j + 512],
                                     start=True, stop=True)
                    nc.scalar.activation(lnr[:, j:j + 512], rrep[:], AF.Ln)
                recip = mha_sb.tile([32, BCH * 256], FP, name="recip")
                nc.scalar.activation(recip[:], lnr[:], AF.Exp, scale=-1.0)
                nc.vector.tensor_tensor(esc[:], esc[:], recip[:], OP.mult)
                # AV: attnT chunk, 2 psum tiles per 4 b's
                for bi in range(0, BCH, 4):
                    for hf in range(2):
                        ap_ps = mha_ps.tile([128, 128], FP, name="ap_ps")
                        for bj in range(4):
                            b = b0 + bi + bj
                            for hq in range(4):
                                hd = hf * 4 + hq
                                col = (bi + bj) * 256 + hd * 32
                                nc.tensor.matmul(
                                    ap_ps[hq * 32:(hq + 1) * 32,
                                          bj * 32:(bj + 1) * 32],
                                    v_tok[0:32, col:col + 32],
                                    esc[0:32, col:col + 32],
                                    start=True, stop=True,
                                    tile_position=(0, hq * 32))
                        nc.vector.tensor_copy(
                            attnT[:, hf * NTOK + (b0 + bi) * 32:
                                  hf * NTOK + (b0 + bi + 4) * 32], ap_ps[:])

        # ---- att_outT + residual -> h1T (bf16 + fp32 copy for later residual)
        h1T = P.tile([128, 2 * NTOK], BF, name="h1T")
        h1Tf = P.tile([128, 2 * NTOK], FP, name="h1Tf")
        for m in range(2):
            for n0 in range(0, NTOK, NCH):
                sl = slice(m * NTOK + n0, m * NTOK + n0 + NCH)
                op_ps = ps.tile([128, NCH], FP, name="op_ps", tag="mmps")
                for k in range(2):
                    nc.tensor.matmul(
                        op_ps[:],
                        opwT[:, k * 256 + m * 128:k * 256 + (m + 1) * 128],
                        attnT[:, k * NTOK + n0:k * NTOK + n0 + NCH],
                        start=(k == 0), stop=(k == 1))
                nc.vector.scalar_tensor_tensor(
                    h1Tf[:, sl], op_ps[:], opb[:, m:m + 1], mlTf[:, sl],
                    OP.add, OP.add)
                nc.scalar.copy(h1T[:, sl], h1Tf[:, sl])

        # ---- MLP fused over n-chunks; outT = tanh(h1T + mlp)
        outT = P.tile([128, 2 * NTOK], FP, name="outT")
        with tc.tile_pool(name="mid_sb", bufs=2) as mid_sb:
            for n0 in range(0, NTOK, NCH):
                mid = mid_sb.tile([128, 8 * NCH], BF, name="mid")
                for m in range(8):
                    mp = ps.tile([128, NCH], FP, name="mp", tag="mmps")
                    for k in range(2):
                        nc.tensor.matmul(
                            mp[:],
                            w1T[:, k * 1024 + m * 128:k * 1024 + (m + 1) * 128],
                            h1T[:, k * NTOK + n0:k * NTOK + n0 + NCH],
                            start=(k == 0), stop=(k == 1))
                    nc.vector.tensor_scalar(
                        mid[:, m * NCH:(m + 1) * NCH], mp[:],
                        b1[:, m:m + 1], 0.0, OP.add, op1=OP.max)
                for m in range(2):
                    op2 = ps.tile([128, NCH], FP, name="op2", tag="mmps")
                    for k in range(8):
                        nc.tensor.matmul(
                            op2[:],
                            w2T[:, k * 256 + m * 128:k * 256 + (m + 1) * 128],
                            mid[:, k * NCH:(k + 1) * NCH],
                            start=(k == 0), stop=(k == 7))
                    pre = mid_sb.tile([128, NCH], FP, name="pre")
                    nc.vector.scalar_tensor_tensor(
                        pre[:], op2[:], b2[:, m:m + 1],
                        h1Tf[:, m * NTOK + n0:m * NTOK + n0 + NCH],
                        OP.add, OP.add)
                    nc.scalar.activation(
                        outT[:, m * NTOK + n0:m * NTOK + n0 + NCH], pre[:],
                        AF.Tanh)

        # ---- pool over s, final head
        pooledT = P.tile([128, 2 * BC], FP, name="pooledT")
        for k in range(2):
            nc.vector.tensor_reduce(
                pooledT[:, k * BC:(k + 1) * BC],
                outT[:, k * NTOK:(k + 1) * NTOK].rearrange(
                    "p (b s) -> p b s", s=NSTOCK),
                AX.X, OP.add)
        fin_ps = ps.tile([BC, 1], FP, name="fin_ps", tag="mmps")
        for k in range(2):
            nc.tensor.matmul(fin_ps[:], pooledT[:, k * BC:(k + 1) * BC],
                             fw[:, k:k + 1],
                             start=(k == 0), stop=(k == 1))
        fin = P.tile([BC, 1], FP, name="fin")
        nc.vector.tensor_copy(fin[:], fin_ps[:])
        nc.sync.dma_start(out=out_d.ap(), in_=fin[:])

    nc.finalize()
    return nc


def _post_core_inputs(cm_b, mc_b, inputs):
    f32 = lambda a: np.ascontiguousarray(np.asarray(a, np.float32))
    mw = float(np.asarray(inputs["macro_weight"]).reshape(-1)[0])
    mean = cm_b.mean(1, keepdims=True)
    std = cm_b.std(1, keepdims=True, ddof=1) + 1e-8
    uT = ((cm_b - mean) / std).T
    vT = (mc_b * mw).T
    ipw = np.asarray(inputs["in_proj_w"], np.float32)
    ipb = np.asarray(inputs["in_proj_b"], np.float32)
    opw = np.asarray(inputs["out_proj_w"], np.float32)
    opb = np.asarray(inputs["out_proj_b"], np.float32)
    qsc = 1.0 / np.sqrt(DH)
    ipwT = ipw.T.copy()
    ipwT[:, 0:H] *= qsc
    ipb_eff = ipb.copy()
    ipb_eff[0:H] *= qsc
    opb_eff = opb + ipb[2 * H:] @ opw.T
    bf16 = lambda a: np.ascontiguousarray(np.asarray(a, np.float32)).astype(BF_NP)
    return {
        "uT": f32(uT),
        "vT": f32(vT),
        "nwT": f32(np.asarray(inputs["norm_weight"]).T),
        "nbT": f32(np.asarray(inputs["norm_bias"]).T),
        "ipwT": bf16(ipwT),
        "ipb": f32(ipb_eff[0:2 * H].reshape(2, 8, 32).transpose(2, 0, 1).reshape(32, 16)),
        "opwT": bf16(opw.T),
        "opb": f32(opb_eff.reshape(2, 128).T),
        "w1T": bf16(np.asarray(inputs["mlp_w1"]).T),
        "b1": f32(np.asarray(inputs["mlp_b1"]).reshape(8, 128).T),
        "w2T": bf16(np.asarray(inputs["mlp_w2"]).T),
        "b2": f32(np.asarray(inputs["mlp_b2"]).reshape(2, 128).T),
        "fw": f32((np.asarray(inputs["final_w"]).reshape(H) / NSTOCK).reshape(H, 1)),
    }


# ----------------------------------------------------------------------------
# host orchestration
# ----------------------------------------------------------------------------

_progs = {}


def run_lstm_launch(inputs, T=S, trace=False):
    if ("lstm", T) not in _progs:
        _progs[("lstm", T)] = build_lstm_program(T)
    nc_a = _progs[("lstm", T)]
    x = np.asarray(inputs["x"], np.float32)
    in_maps = []
    for core in range(8):
        q = core % 4
        xb = x[q * BL:(q + 1) * BL]
        if core < 4:
            m = _lstm_core_inputs(xb, inputs["stock_tr_w"], inputs["stock_tr_b"],
                                  inputs["s_wih"], inputs["s_whh"],
                                  inputs["s_bih"], inputs["s_bhh"], T)
        else:
            m = _lstm_core_inputs(xb, inputs["macro_tr_w"], inputs["macro_tr_b"],
                                  inputs["m_wih"], inputs["m_whh"],
                                  inputs["m_bih"], inputs["m_bhh"], T)
        in_maps.append(m)
    res = run_bass_kernel_spmd(nc_a, in_maps, core_ids=list(range(8)),
                               trace=trace)
    def ctx_of(i):
        return res.results[i]["ctx"] / res.results[i]["den"]
    c_matrix = np.concatenate([ctx_of(i) for i in range(4)], 0)
    macro_ctx = np.concatenate([ctx_of(i) for i in range(4, 8)], 0)
    return c_matrix, macro_ctx, res


def run_post_launch(c_matrix, macro_ctx, inputs, trace=False):
    if "post" not in _progs:
        _progs["post"] = build_post_program()
    nc_b = _progs["post"]
    in_maps = [
        _post_core_inputs(c_matrix[c * BC:(c + 1) * BC],
                          macro_ctx[c * BC:(c + 1) * BC], inputs)
        for c in range(8)
    ]
    res = run_bass_kernel_spmd(nc_b, in_maps, core_ids=list(range(8)),
                               trace=trace)
    fb = float(np.asarray(inputs["final_b"]).reshape(-1)[0])
    out = np.concatenate(
        [res.results[i]["out"].reshape(BC) for i in range(8)], 0) + fb
    return out.astype(np.float32), res


def kernel(**inputs):
    c_matrix, macro_ctx, _ = run_lstm_launch(inputs)
    out, _ = run_post_launch(c_matrix, macro_ctx, inputs)
    return out

